# revision 40
# baseline (speedup 1.0000x reference)
"""Fused GPT transformer layer on 8 trn2 cores — token-parallel + KV AllGather.

Sharding: core i owns 512 contiguous tokens (cores 0-3 batch 0, 4-7 batch 1).
Per core: LN1 -> QKV (+RoPE) local; AllGather K^T (bf16) and V (fp8) within
4-core group; masked full-key attention (softmax without max-subtraction);
dense+residual, LN2, fused chunked MLP all local. Host gathers outputs.

v4: all big GEMMs (QKV, V, dense, MLP1, MLP2) run fp8e4 DoubleRow (K=256 per
matmul, 2x FLOP/instr at the same 220ns/MM issue rate). Weights pre-scaled
x32 into fp8 on host; descale 1/32 folded into PSUM-evacuation activations.
Attention scores stay bf16; probs+V are fp8 so context & softmax-denominator
matmuls are DoubleRow too. Q/K bias folded into scalar.activation (T-layout
per-partition bias). Order K -> AG(K) -> V -> AG(V) -> Q -> attention so
local compute hides both collectives. Hidden state kept in SBUF (no DRAM
bounce). Weights pre-arranged on host for contiguous per-partition DMA.

Layouts: "N" = [token-partition, feature-free]; "T" = [feature-part, token-free].
fp8 3D tiles [128, k-block, free] feed DoubleRow pairs [:, 2k:2k+2, :].
"""
import sys
if '/opt/trn_rl_repo' not in sys.path:
    sys.path.insert(0, '/opt/trn_rl_repo')

from dataclasses import dataclass

import numpy as np
import ml_dtypes

import concourse.bass as bass
import concourse.bacc as bacc
import concourse.tile as tile
import concourse.mybir as mybir
from concourse import bass_utils
from concourse.masks import make_identity
from concourse.replica_groups import maybe_share_collective_output_space

F32 = mybir.dt.float32
BF16 = mybir.dt.bfloat16
FP8 = mybir.dt.float8e4
AF = mybir.ActivationFunctionType
ALU = mybir.AluOpType
DR = mybir.MatmulPerfMode.DoubleRow
BF = ml_dtypes.bfloat16
E4 = ml_dtypes.float8_e4m3fn
SW = 1.0    # bf16 weights: no pre-scale


@dataclass
class Cfg:
    B: int = 2
    S: int = 2048
    H: int = 2048
    NH: int = 16
    FF: int = 8192
    W: int = 8           # total cores
    FC: int = 1024       # FF chunk for fused MLP
    WSP: int = 512       # weight panel span (moving free dim for N-layout mms)
    EPS: float = 1e-5
    phase_limit: int = 99   # 1=A, 2=B(+AG), 3=C, 4=D, 5=E

    @property
    def HD(self):
        return self.H // self.NH

    @property
    def NG(self):
        return self.W // self.B

    @property
    def T(self):
        return self.S // self.NG

    @property
    def NT(self):
        return self.T // 128

    @property
    def KH(self):
        return self.H // 128

    @property
    def NOS(self):
        return self.H // self.WSP

    @property
    def SCALE(self):
        return 1.0 / float(np.sqrt(self.HD))


def build(cfg: Cfg):
    c = cfg
    assert c.HD == 128 and c.T % 128 == 0 and c.H % c.WSP == 0
    assert c.FF % c.FC == 0 and c.FC % 128 == 0

    nc = bacc.Bacc("TRN2", target_bir_lowering=False, debug=False,
                   num_devices=c.W)
    d = lambda name, shape, dt=F32: nc.dram_tensor(name, shape, dt,
                                                   kind="ExternalInput")
    io = {}
    io["x_in"] = d("x", [c.T, c.H])
    io["wq_in"] = d("wq", [c.NH * 128, c.KH * 128], BF16)
    io["wk_in"] = d("wk", [c.NH * 128, c.KH * 128], BF16)
    io["wv_in"] = d("wv", [128, c.KH * c.H], BF16)
    io["wd_in"] = d("wd", [128, c.KH * c.H], BF16)
    io["w1_in"] = d("w1", [c.FF, c.KH * 128], BF16)
    io["w2_in"] = d("w2", [(c.FF // c.FC) * 128, (c.FC // 128) * c.H], BF16)
    io["bq_in"] = d("bq", [128, c.NH])
    io["bk_in"] = d("bk", [128, c.NH])
    io["bd_in"] = d("bd", [1, c.H], BF16)   # SW*(bv@wd + b_dense)
    io["b1_in"] = d("b1", [128, c.FF // 128])
    io["b2_in"] = d("b2", [1, c.H], BF16)   # SW*b2
    io["ones_r_in"] = d("ones_r", [1, c.T], BF16)
    io["onesdr_in"] = d("onesdr", [128, 32], FP8)
    io["cos_in"] = d("cosT", [128, c.T])
    io["sins_in"] = d("sinsT", [128, c.T])
    io["mask_in"] = d("maskT", [c.S, c.T], BF16)   # lm-major key blocks
    io["out_ext"] = nc.dram_tensor("out", [c.T, c.H], F32, kind="ExternalOutput")
    io["groups"] = [[g * c.NG + r for r in range(c.NG)] for g in range(c.B)]

    with tile.TileContext(nc) as tc:
        _body(nc, tc, c, io)
    nc.compile()
    return nc


def _body(nc, tc, c, io):
    x_in, out_ext = io["x_in"], io["out_ext"]
    NT, KH, NH, T, H = c.NT, c.KH, c.NH, c.T, c.H
    WSP, NOS, NG = c.WSP, c.NOS, c.NG
    KH2 = KH // 2
    SKT = c.S // 128
    AXX = mybir.AxisListType.X
    DSC = 1.0 / SW

    # ---------- persistent pools ----------
    const = tc.alloc_tile_pool(name="const", bufs=1)
    ident = const.tile([128, 128], F32, tag="ident", name="ident")
    make_identity(nc, ident[:])
    ones_r = const.tile([1, T], BF16, tag="ones_r", name="ones_r")
    nc.sync.dma_start(ones_r[:], io["ones_r_in"].ap()[:])
    onesdr = const.tile([128, 2, 16], FP8, tag="onesdr", name="onesdr")
    nc.sync.dma_start(onesdr[:].rearrange("p a b -> p (a b)"),
                      io["onesdr_in"].ap()[:])
    epsap = const.tile([128, 1], F32, tag="epsap", name="epsap")
    nc.gpsimd.memset(epsap[:], c.EPS)
    eshift = const.tile([128, 1], F32, tag="eshift", name="eshift")
    nc.gpsimd.memset(eshift[:], -1.5)
    b1_sb = const.tile([128, c.FF // 128], F32, tag="b1", name="b1")
    nc.sync.dma_start(b1_sb[:], io["b1_in"].ap()[:])
    bq_sb = const.tile([128, NH], F32, tag="bq", name="bq")
    nc.sync.dma_start(bq_sb[:], io["bq_in"].ap()[:])
    bk_sb = const.tile([128, NH], F32, tag="bk", name="bk")
    nc.sync.dma_start(bk_sb[:], io["bk_in"].ap()[:])
    cos_sb = const.tile([128, T], F32, tag="cos", name="cos")
    nc.sync.dma_start(cos_sb[:], io["cos_in"].ap()[:])
    sins_sb = const.tile([128, T], F32, tag="sins", name="sins")
    nc.sync.dma_start(sins_sb[:], io["sins_in"].ap()[:])
    stat = tc.alloc_tile_pool(name="stat", bufs=2)
    big = tc.alloc_tile_pool(name="big", bufs=1)
    pp = tc.alloc_tile_pool(name="pp", bufs=1, space="PSUM")
    dram = tc.alloc_tile_pool(name="dram", bufs=1, space="DRAM")

    kT_bounce = dram.tile([H, T], BF16, tag="kTb", name="kTb")
    hid_b = dram.tile([c.T, H], F32, tag="hidb", name="hidb")
    ag_space = maybe_share_collective_output_space("AllGather", io["groups"])
    kT_all = dram.tile([NG * H, T], BF16, tag="kTall", name="kTall",
                       addr_space=ag_space)
    v_bnc = [dram.tile([128, H], FP8, tag="vb", name=f"vb{t}", bufs=NT)
             for t in range(NT)]
    v_alls = [dram.tile([NG * 128, H], FP8, tag="vall", name=f"vall{t}",
                        bufs=NT, addr_space=ag_space) for t in range(NT)]

    NCH = H // 512

    def ln_tile(src, out):
        """LN stats + normalize for one N-layout tile [128, H] -> out."""
        stats = stat.tile([128, NCH, 6], F32, tag="bnst", name="bnst")
        srcr = src[:].rearrange("p (n f) -> p n f", f=512)
        for ch in range(NCH):
            nc.vector.bn_stats(stats[:, ch, :], srcr[:, ch, :])
        mv = stat.tile([128, 2], F32, tag="mv", name="mv")
        nc.vector.bn_aggr(mv[:], stats[:])
        std = stat.tile([128, 1], F32, tag="std", name="std")
        nc.scalar.activation(std[:], mv[:, 1:2], AF.Sqrt, bias=epsap[:],
                             scale=1.0)
        rstd = stat.tile([128, 1], F32, tag="rstd", name="rstd")
        nc.vector.reciprocal(rstd[:], std[:])
        negmr = stat.tile([128, 1], F32, tag="negmr", name="negmr")
        nc.vector.scalar_tensor_tensor(negmr[:], mv[:, 0:1], -1.0, rstd[:],
                                       op0=ALU.mult, op1=ALU.mult)
        nc.scalar.activation(out[:], src[:], AF.Identity,
                             bias=negmr[:], scale=rstd[:])

    def transpose_tile(srcN, dst8, t):
        """[128tok, H] f32 -> cast into fp8 T-layout tile dst8 at column t."""
        for kk in range(KH):
            ps = pp.tile([128, 128], F32, tag="ptr", name="ptr", bufs=2)
            nc.tensor.transpose(ps[:], srcN[:, 128 * kk:128 * (kk + 1)], ident[:])
            nc.vector.tensor_copy(dst8[:, kk:kk + 1, 128 * t:128 * (t + 1)],
                                  ps[:])

    # ---------- Phase A: LN1 + transpose (x streamed) ----------
    xT8 = big.tile([128, KH, T], BF16, tag="TT8", name="xT8", bufs=1)
    for t in range(NT):
        xt = big.tile([128, H], F32, tag="bigH", name=f"x{t}", bufs=4)
        nc.sync.dma_start(xt[:], x_in.ap()[128 * t:128 * (t + 1), :])
        xh = big.tile([128, H], F32, tag="bigH", name=f"xh{t}", bufs=4)
        ln_tile(xt, xh)
        transpose_tile(xh, xT8, t)

    def bail(*pools):
        for p in pools:
            p.release()

    if c.phase_limit <= 1:
        bail(pp, dram, big, stat, const)
        return

    # ---------- Phase B: K -> AG(K); V -> AG(V); Q ----------
    pb = tc.alloc_tile_pool(name="pb", bufs=1)
    # prefetch V weights: needed right after the K heads
    wv_sb = pb.tile([128, KH, H], BF16, tag="wv", name="wv")
    for q4 in range(4):
        nc.sync.dma_start(
            wv_sb[:, 4 * q4:4 * (q4 + 1), :].rearrange("p a b -> p (a b)"),
            io["wv_in"].ap()[:, 4 * q4 * H:4 * (q4 + 1) * H])
    qT = [big.tile([128, T], BF16, tag="qT", name=f"qT{h}", bufs=NH)
          for h in range(NH)]

    def qk_head(h, w_in, b_sb, dst):
        """dst: (dram_ap, row0) or sbuf bf16 tile [128, T]."""
        ps = pp.tile([128, T], F32, tag="mm", name="pqk", bufs=2)
        wt = pb.tile([128, KH, 128], BF16, tag="wqk", name="wqk", bufs=4)
        nc.sync.dma_start(wt[:].rearrange("p a b -> p (a b)"),
                          w_in.ap()[128 * h:128 * (h + 1), :])
        for kk in range(KH):
            nc.tensor.matmul(ps[:], wt[:, kk:kk + 1, :],
                             xT8[:, kk:kk + 1, :],
                             start=(kk == 0), stop=(kk == KH - 1))
        psb = pb.tile([128, T], F32, tag="psb", name="psb", bufs=2)
        nc.scalar.activation(psb[:], ps[:], AF.Identity,
                             bias=b_sb[:, h:h + 1], scale=DSC)
        # partition-swapped biased copy (rotate_half operand), read from PSUM
        psw = pb.tile([128, T], F32, tag="psw", name="psw", bufs=2)
        nc.scalar.activation(psw[0:64, :], ps[64:128, :], AF.Identity,
                             bias=b_sb[64:128, h:h + 1], scale=DSC)
        nc.scalar.activation(psw[64:128, :], ps[0:64, :], AF.Identity,
                             bias=b_sb[0:64, h:h + 1], scale=DSC)
        tmp = pb.tile([128, T], F32, tag="ropetmp", name="ropetmp", bufs=2)
        nc.vector.scalar_tensor_tensor(tmp[:], psw[:], 1.0, sins_sb[:],
                                       op0=ALU.mult, op1=ALU.mult)
        qc = pb.tile([128, T], F32, tag="ropeqc", name="ropeqc", bufs=2)
        nc.vector.scalar_tensor_tensor(qc[:], psb[:], 1.0, cos_sb[:],
                                       op0=ALU.mult, op1=ALU.mult)
        if isinstance(dst, tuple):
            res = pb.tile([128, T], BF16, tag="qkres", name="qkres", bufs=3)
            nc.vector.scalar_tensor_tensor(res[:], qc[:], 1.0, tmp[:],
                                           op0=ALU.mult, op1=ALU.add)
            d_ap, row0 = dst
            nc.sync.dma_start(d_ap[row0:row0 + 128, :], res[:])
        else:
            nc.vector.scalar_tensor_tensor(dst[:], qc[:], 1.0, tmp[:],
                                           op0=ALU.mult, op1=ALU.add)

    for h in range(NH):
        qk_head(h, io["wk_in"], bk_sb, (kT_bounce, 128 * h))
    nc.gpsimd.collective_compute(
        "AllGather", ALU.bypass, ins=[kT_bounce.opt()], outs=[kT_all.opt()],
        replica_groups=io["groups"])

    # V: N-layout DoubleRow; v kept at x32 scale in fp8 (descale in softmax)
    ppv = tc.alloc_tile_pool(name="ppv", bufs=1, space="PSUM")
    for t in range(NT):
        pss = [ppv.tile([128, WSP], F32, tag=f"pvac{o}", name=f"pvac{o}",
                        bufs=1) for o in range(NOS)]
        for kk in range(KH):
            lhs = xT8[:, kk:kk + 1, 128 * t:128 * (t + 1)]
            for osp in range(NOS):
                nc.tensor.matmul(pss[osp][:], lhs,
                                 wv_sb[:, kk:kk + 1,
                                       WSP * osp:WSP * (osp + 1)],
                                 start=(kk == 0), stop=(kk == KH - 1))
        for osp in range(NOS):
            vs = pb.tile([128, WSP], FP8, tag="vslice", name="vslice", bufs=4)
            nc.vector.tensor_copy(vs[:], pss[osp][:])
            nc.sync.dma_start(v_bnc[t][:, WSP * osp:WSP * (osp + 1)], vs[:])
        nc.gpsimd.collective_compute(
            "AllGather", ALU.bypass, ins=[v_bnc[t].opt()],
            outs=[v_alls[t].opt()], replica_groups=io["groups"])
    ppv.release()

    for h in range(NH):
        qk_head(h, io["wq_in"], bq_sb, qT[h])
    pb.release()

    if c.phase_limit <= 2:
        bail(pp, dram, big, stat, const)
        return

    # ---------- Phase C: attention ----------
    # release pp (frees ptr+mm banks) and prefetch dense weights
    pp.release()
    pcd = tc.alloc_tile_pool(name="pcd", bufs=1)
    wd_sb = pcd.tile([128, KH, H], BF16, tag="wd", name="wd")
    for q4 in range(4):
        nc.sync.dma_start(
            wd_sb[:, 4 * q4:4 * (q4 + 1), :].rearrange("p a b -> p (a b)"),
            io["wd_in"].ap()[:, 4 * q4 * H:4 * (q4 + 1) * H])
    pc = tc.alloc_tile_pool(name="pc", bufs=1)
    ppc = tc.alloc_tile_pool(name="ppc", bufs=1, space="PSUM")
    mask_sb = [pcd.tile([128, T], BF16, tag="mask", name=f"mask{m}", bufs=SKT)
               for m in range(SKT)]
    for m in range(SKT):
        nc.sync.dma_start(mask_sb[m][:],
                          io["mask_in"].ap()[128 * m:128 * (m + 1), :])
    ctx8 = pcd.tile([128, NH, T], BF16, tag="ctx8", name="ctx8")
    for h in range(NH):
        kpan = pc.tile([128, NG * T], BF16, tag="kpan", name="kpan", bufs=2)
        for rnk in range(NG):
            nc.sync.dma_start(
                kpan[:, rnk * T:(rnk + 1) * T],
                kT_all[rnk * H + 128 * h:rnk * H + 128 * (h + 1), :])
        vpan8 = pc.tile([128, SKT, 128], FP8, tag="vpan8", name="vpan8",
                        bufs=2)
        for lm in range(NT):
            nc.sync.dma_start(
                vpan8[:, lm * NG:(lm + 1) * NG, :],
                v_alls[lm].rearrange("(r p) cc -> p r cc", p=128)
                [:, :, 128 * h:128 * (h + 1)])
        ems8 = pc.tile([128, SKT, T], FP8, tag="ems8", name="ems8", bufs=1)
        for g in range(SKT):
            lm, rnk = g // NG, g % NG
            ps_s = ppc.tile([128, T], F32, tag="pscore", name="pscore", bufs=2)
            nc.tensor.matmul(
                ps_s[:], kpan[:, rnk * T + 128 * lm:rnk * T + 128 * (lm + 1)],
                qT[h][:], start=True, stop=True)
            e_m = pc.tile([128, T], BF16, tag="eatt", name="eatt", bufs=4)
            # -1.5 shift keeps exp under fp8e4 max (448) for scores < 7.6
            # sigma; cancels in the softmax normalization.
            nc.scalar.activation(e_m[:], ps_s[:], AF.Exp, bias=eshift[:],
                                 scale=c.SCALE)
            nc.vector.scalar_tensor_tensor(ems8[:, g:g + 1, :], e_m[:], 1.0,
                                           mask_sb[g][:], op0=ALU.mult,
                                           op1=ALU.mult)
        ps_ctx = ppc.tile([128, T], F32, tag="pctx", name="pctx", bufs=2)
        ps_sum = ppc.tile([16, T], F32, tag="psml", name="psml", bufs=1)
        for j in range(SKT // 2):
            nc.tensor.matmul(ps_ctx[:], vpan8[:, 2 * j:2 * j + 2, :],
                             ems8[:, 2 * j:2 * j + 2, :],
                             start=(j == 0), stop=(j == SKT // 2 - 1),
                             perf_mode=DR)
        for j in range(SKT // 2):
            nc.tensor.matmul(ps_sum[:], onesdr[:],
                             ems8[:, 2 * j:2 * j + 2, :],
                             start=(j == 0), stop=(j == SKT // 2 - 1),
                             perf_mode=DR)
        rsum = stat.tile([1, T], F32, tag="rsum", name="rsum")
        nc.vector.reciprocal(rsum[:], ps_sum[0:1, :])
        rrep = stat.tile([128, T], F32, tag="rsumrep", name="rsumrep")
        nc.gpsimd.partition_broadcast(rrep[:], rsum[:])
        nc.vector.scalar_tensor_tensor(ctx8[:, h:h + 1, :], ps_ctx[:], DSC,
                                       rrep[:], op0=ALU.mult, op1=ALU.mult)
    ppc.release()
    pc.release()

    if c.phase_limit <= 3:
        bail(pcd, dram, big, stat, const)
        return

    # ---------- Phase D: dense + residual, LN2, transpose ----------
    pd = tc.alloc_tile_pool(name="pd", bufs=1)
    ppd = tc.alloc_tile_pool(name="ppd", bufs=1, space="PSUM")
    bts = []
    for osp in range(NOS):
        bt = pd.tile([1, WSP], BF16, tag="bdsl", name="bdsl", bufs=NOS)
        nc.sync.dma_start(bt[:], io["bd_in"].ap()[:, WSP * osp:WSP * (osp + 1)])
        bts.append(bt)
    for t in range(NT):
        pss = [ppd.tile([128, WSP], F32, tag=f"pdac{o}", name=f"pdac{o}",
                        bufs=1) for o in range(NOS)]
        for kk in range(KH):
            lhs = ctx8[:, kk:kk + 1, 128 * t:128 * (t + 1)]
            for osp in range(NOS):
                nc.tensor.matmul(pss[osp][:], lhs,
                                 wd_sb[:, kk:kk + 1,
                                       WSP * osp:WSP * (osp + 1)],
                                 start=(kk == 0), stop=False)
        for osp in range(NOS):
            nc.tensor.matmul(pss[osp][:], ones_r[:, 0:128], bts[osp][:],
                             start=False, stop=True)
            xs = pd.tile([128, WSP], F32, tag="xsl", name="xsl", bufs=3)
            nc.sync.dma_start(
                xs[:], x_in.ap()[128 * t:128 * (t + 1),
                                 WSP * osp:WSP * (osp + 1)])
            hs = pd.tile([128, WSP], F32, tag="hsl", name="hsl", bufs=3)
            nc.vector.scalar_tensor_tensor(hs[:], pss[osp][:], DSC,
                                           xs[:], op0=ALU.mult, op1=ALU.add)
            nc.sync.dma_start(
                hid_b[128 * t:128 * (t + 1), WSP * osp:WSP * (osp + 1)], hs[:])
    ppd.release()
    pd.release()
    pcd.release()

    # LN2 + transpose (pp re-alloc for transposes + MLP1 psums)
    pp2 = tc.alloc_tile_pool(name="pp2", bufs=1, space="PSUM")
    hT8 = big.tile([128, KH, T], BF16, tag="TT8", name="hT8", bufs=1)
    for t in range(NT):
        ht = big.tile([128, H], F32, tag="bigH", name=f"hid{t}", bufs=4)
        nc.sync.dma_start(ht[:], hid_b[128 * t:128 * (t + 1), :])
        hh = big.tile([128, H], F32, tag="bigH", name=f"hh{t}", bufs=4)
        ln_tile(ht, hh)
        for kk in range(KH):
            ps = pp2.tile([128, 128], F32, tag="ptr2", name="ptr2", bufs=2)
            nc.tensor.transpose(ps[:], hh[:, 128 * kk:128 * (kk + 1)], ident[:])
            nc.vector.tensor_copy(hT8[:, kk:kk + 1, 128 * t:128 * (t + 1)],
                                  ps[:])

    if c.phase_limit <= 4:
        bail(pp2, dram, big, stat, const)
        return

    # ---------- Phase E: fused MLP ----------
    pe = tc.alloc_tile_pool(name="pe", bufs=1)
    ppe2 = tc.alloc_tile_pool(name="ppe2", bufs=1, space="PSUM")
    NFC = c.FF // c.FC
    FCT = c.FC // 128
    out_t = [big.tile([128, H], F32, tag="bigH", name=f"out{t}", bufs=4)
             for t in range(NT)]
    for f in range(NFC):
        g8 = pe.tile([128, FCT, T], BF16, tag="g8", name="g8", bufs=2)
        for mm in range(FCT):
            fglob = f * FCT + mm
            w1t = pe.tile([128, KH, 128], BF16, tag="w1pan", name="w1pan",
                          bufs=4)
            nc.sync.dma_start(w1t[:].rearrange("p a b -> p (a b)"),
                              io["w1_in"].ap()[128 * fglob:128 * (fglob + 1), :])
            ps = pp2.tile([128, T], F32, tag="mm1", name="pm1", bufs=2)
            for kk in range(KH):
                nc.tensor.matmul(ps[:], w1t[:, kk:kk + 1, :],
                                 hT8[:, kk:kk + 1, :],
                                 start=(kk == 0), stop=(kk == KH - 1))
            nc.scalar.activation(g8[:, mm:mm + 1, :], ps[:], AF.Gelu,
                                 bias=b1_sb[:, fglob:fglob + 1], scale=DSC)
        w2t = pe.tile([128, FCT, H], BF16, tag="w2pan", name="w2pan", bufs=2)
        for q2 in range(2):
            hfc = FCT // 2
            nc.sync.dma_start(
                w2t[:, hfc * q2:hfc * (q2 + 1), :].rearrange("p a b -> p (a b)"),
                io["w2_in"].ap()[128 * f:128 * (f + 1),
                                 hfc * q2 * H:hfc * (q2 + 1) * H])
        if f == 0:
            b2s = []
            for osp in range(NOS):
                bt = pe.tile([1, WSP], BF16, tag="b2sl", name="b2sl", bufs=NOS)
                nc.sync.dma_start(
                    bt[:], io["b2_in"].ap()[:, WSP * osp:WSP * (osp + 1)])
                b2s.append(bt)
        for t in range(NT):
            pss = [ppe2.tile([128, WSP], F32, tag=f"pmac{o}", name=f"pmac{o}",
                             bufs=1) for o in range(NOS)]
            for kf in range(FCT):
                lhs = g8[:, kf:kf + 1, 128 * t:128 * (t + 1)]
                for osp in range(NOS):
                    nc.tensor.matmul(pss[osp][:], lhs,
                                     w2t[:, kf:kf + 1,
                                         WSP * osp:WSP * (osp + 1)],
                                     start=(kf == 0),
                                     stop=(kf == FCT - 1 and f != 0))
            for osp in range(NOS):
                osl = out_t[t][:, WSP * osp:WSP * (osp + 1)]
                if f == 0:
                    nc.tensor.matmul(pss[osp][:], ones_r[:, 0:128], b2s[osp][:],
                                     start=False, stop=True)
                    hsl = pe.tile([128, WSP], F32, tag="hres", name="hres",
                                  bufs=3)
                    nc.sync.dma_start(
                        hsl[:], hid_b[128 * t:128 * (t + 1),
                                      WSP * osp:WSP * (osp + 1)])
                    nc.vector.scalar_tensor_tensor(osl, pss[osp][:], DSC,
                                                   hsl[:], op0=ALU.mult,
                                                   op1=ALU.add)
                else:
                    nc.vector.scalar_tensor_tensor(osl, pss[osp][:], DSC, osl,
                                                   op0=ALU.mult, op1=ALU.add)
    ppe2.release()
    pe.release()

    # ---------- Phase F: output ----------
    for t in range(NT):
        nc.sync.dma_start(out_ext.ap()[128 * t:128 * (t + 1), :], out_t[t][:])

    for p in (pp2, dram, big, stat, const):
        p.release()


# ---------------- host side ----------------

def prepare_in_maps(c: Cfg, inputs):
    f32 = np.float32
    hs = np.asarray(inputs["hidden_states"], f32)
    ln1_g = np.asarray(inputs["ln1_g"], f32)
    ln1_b = np.asarray(inputs["ln1_b"], f32)
    w_qkv = np.asarray(inputs["w_qkv"], f32)
    b_qkv = np.asarray(inputs["b_qkv"], f32)
    w_dense = np.asarray(inputs["w_dense"], f32)
    b_dense = np.asarray(inputs["b_dense"], f32)
    ln2_g = np.asarray(inputs["ln2_g"], f32)
    ln2_b = np.asarray(inputs["ln2_b"], f32)
    w1 = np.asarray(inputs["w1"], f32)
    b1 = np.asarray(inputs["b1"], f32)
    w2 = np.asarray(inputs["w2"], f32)
    b2 = np.asarray(inputs["b2"], f32)

    H, NH, HD, FF, KH = c.H, c.NH, c.HD, c.FF, c.KH
    NFC, FCT = FF // c.FC, c.FC // 128
    cols = np.concatenate([np.arange(h * 3 * HD, h * 3 * HD + HD)
                           for h in range(NH)])
    wg = ln1_g[:, None] * w_qkv
    wq_f, wk_f, wv_f = wg[:, cols], wg[:, cols + HD], wg[:, cols + 2 * HD]
    bfull = ln1_b @ w_qkv + b_qkv
    bq_f, bk_f, bv_f = bfull[cols], bfull[cols + HD], bfull[cols + 2 * HD]
    bd_f = bv_f @ w_dense + b_dense          # v-bias folded through attention
    w1_f = ln2_g[:, None] * w1
    b1_f = ln2_b @ w1 + b1

    # head-major rows [NH*128, KH*128]: block h = weights for head h
    hmaj = lambda w, nb: np.ascontiguousarray(
        w.reshape(KH, 128, nb, 128).transpose(2, 1, 0, 3)
        .reshape(nb * 128, KH * 128).astype(BF))
    # [128, KH*H]: row p, col kk*H + cc  =  w[kk*128+p, cc]
    parr = lambda w: np.ascontiguousarray(
        w.reshape(KH, 128, H).transpose(1, 0, 2)
        .reshape(128, KH * H).astype(BF))
    wqh = hmaj(wq_f, NH)
    wkh = hmaj(wk_f, NH)
    wvh = parr(wv_f)
    wdh = parr(w_dense)
    w1h = hmaj(w1_f, FF // 128)
    w2h = np.ascontiguousarray(
        w2.reshape(NFC, FCT, 128, H).transpose(0, 2, 1, 3)
        .reshape(NFC * 128, FCT * H).astype(BF))

    inv = 1.0 / (10000.0 ** (np.arange(0, HD, 2, dtype=f32) / HD))
    pos = np.arange(c.S, dtype=f32)
    frq = np.einsum('i,j->ij', pos, inv)
    emb = np.concatenate([frq, frq], axis=-1)
    cos_full = np.cos(emb).T.astype(f32)
    sin_full = np.sin(emb).T.astype(f32)
    sins_full = sin_full.copy()
    sins_full[:HD // 2] *= -1.0

    bf = lambda a: np.ascontiguousarray(a.astype(BF))
    in_maps = []
    for i in range(c.W):
        b, g = i // c.NG, i % c.NG
        t0 = g * c.T
        qpos = np.arange(t0, t0 + c.T)
        # lm-major mask: slot gp = lm*NG + rnk covers key block rnk*NT + lm
        mrows = []
        for gp in range(c.S // 128):
            lm, rnk = gp // c.NG, gp % c.NG
            kb = rnk * c.NT + lm
            kpos = np.arange(kb * 128, (kb + 1) * 128)
            mrows.append((kpos[:, None] <= qpos[None, :]).astype(BF))
        mask = np.concatenate(mrows, axis=0)
        in_maps.append({
            "x": np.ascontiguousarray(hs[b, t0:t0 + c.T, :]),
            "wq": wqh, "wk": wkh, "wv": wvh, "wd": wdh, "w1": w1h, "w2": w2h,
            "bq": np.ascontiguousarray(bq_f.reshape(NH, 128).T.astype(f32)),
            "bk": np.ascontiguousarray(bk_f.reshape(NH, 128).T.astype(f32)),
            "bd": bf(SW * bd_f.reshape(1, H)),
            "b1": np.ascontiguousarray(b1_f.reshape(FF // 128, 128).T
                                       .astype(f32)),
            "b2": bf(SW * b2.reshape(1, H)),
            "ones_r": np.ones((1, c.T), BF),
            "onesdr": np.ones((128, 32), E4),
            "cosT": np.ascontiguousarray(cos_full[:, t0:t0 + c.T]),
            "sinsT": np.ascontiguousarray(sins_full[:, t0:t0 + c.T]),
            "maskT": np.ascontiguousarray(mask),
        })
    return in_maps


def assemble_output(c: Cfg, results):
    out = np.empty((c.B, c.S, c.H), np.float32)
    for i in range(c.W):
        b, g = i // c.NG, i % c.NG
        out[b, g * c.T:(g + 1) * c.T, :] = results[i]["out"]
    return out


def run(nc, c: Cfg, inputs, trace=False, **kw):
    in_maps = prepare_in_maps(c, inputs)
    last = None
    for attempt in range(3):
        try:
            res = bass_utils.run_bass_kernel_spmd(
                nc, in_maps, core_ids=list(range(c.W)), trace=trace, **kw)
            return assemble_output(c, res.results), res
        except Exception as e:
            last = e
            print(f"run attempt {attempt} failed: {type(e).__name__}: {e}",
                  file=sys.stderr)
    raise last


# ======================= harness entry point =======================

_CACHE = {}


def kernel(**inputs):
    """Full-input entry: shard, compile (cached), run on 8 cores, gather."""
    c = Cfg()
    if "nc" not in _CACHE:
        _CACHE["nc"] = build(c)
    out, _ = run(_CACHE["nc"], c, inputs, trace=False)
    return out


# revision 41
# speedup vs baseline: 1.0158x; 1.0158x over previous
"""Fused GPT transformer layer on 8 trn2 cores — token-parallel + KV AllGather.

Sharding: core i owns 512 contiguous tokens (cores 0-3 batch 0, 4-7 batch 1).
Per core: LN1 -> QKV (+RoPE) local; AllGather K^T (bf16) and V (fp8) within
4-core group; masked full-key attention (softmax without max-subtraction);
dense+residual, LN2, fused chunked MLP all local. Host gathers outputs.

v4: all big GEMMs (QKV, V, dense, MLP1, MLP2) run fp8e4 DoubleRow (K=256 per
matmul, 2x FLOP/instr at the same 220ns/MM issue rate). Weights pre-scaled
x32 into fp8 on host; descale 1/32 folded into PSUM-evacuation activations.
Attention scores stay bf16; probs+V are fp8 so context & softmax-denominator
matmuls are DoubleRow too. Q/K bias folded into scalar.activation (T-layout
per-partition bias). Order K -> AG(K) -> V -> AG(V) -> Q -> attention so
local compute hides both collectives. Hidden state kept in SBUF (no DRAM
bounce). Weights pre-arranged on host for contiguous per-partition DMA.

Layouts: "N" = [token-partition, feature-free]; "T" = [feature-part, token-free].
fp8 3D tiles [128, k-block, free] feed DoubleRow pairs [:, 2k:2k+2, :].
"""
import sys
if '/opt/trn_rl_repo' not in sys.path:
    sys.path.insert(0, '/opt/trn_rl_repo')

from dataclasses import dataclass

import numpy as np
import ml_dtypes

import concourse.bass as bass
import concourse.bacc as bacc
import concourse.tile as tile
import concourse.mybir as mybir
from concourse import bass_utils
from concourse.masks import make_identity
from concourse.replica_groups import maybe_share_collective_output_space

F32 = mybir.dt.float32
BF16 = mybir.dt.bfloat16
FP8 = mybir.dt.float8e4
AF = mybir.ActivationFunctionType
ALU = mybir.AluOpType
DR = mybir.MatmulPerfMode.DoubleRow
BF = ml_dtypes.bfloat16
E4 = ml_dtypes.float8_e4m3fn
SW = 1.0    # bf16 weights: no pre-scale


@dataclass
class Cfg:
    B: int = 2
    S: int = 2048
    H: int = 2048
    NH: int = 16
    FF: int = 8192
    W: int = 8           # total cores
    FC: int = 1024       # FF chunk for fused MLP
    WSP: int = 512       # weight panel span (moving free dim for N-layout mms)
    EPS: float = 1e-5
    phase_limit: int = 99   # 1=A, 2=B(+AG), 3=C, 4=D, 5=E

    @property
    def HD(self):
        return self.H // self.NH

    @property
    def NG(self):
        return self.W // self.B

    @property
    def T(self):
        return self.S // self.NG

    @property
    def NT(self):
        return self.T // 128

    @property
    def KH(self):
        return self.H // 128

    @property
    def NOS(self):
        return self.H // self.WSP

    @property
    def SCALE(self):
        return 1.0 / float(np.sqrt(self.HD))


def build(cfg: Cfg):
    c = cfg
    assert c.HD == 128 and c.T % 128 == 0 and c.H % c.WSP == 0
    assert c.FF % c.FC == 0 and c.FC % 128 == 0

    nc = bacc.Bacc("TRN2", target_bir_lowering=False, debug=False,
                   num_devices=c.W)
    d = lambda name, shape, dt=F32: nc.dram_tensor(name, shape, dt,
                                                   kind="ExternalInput")
    io = {}
    io["x_in"] = d("x", [c.T, c.H])
    io["wq_in"] = d("wq", [c.NH * 128, c.KH * 128], BF16)
    io["wk_in"] = d("wk", [c.NH * 128, c.KH * 128], BF16)
    io["wv_in"] = d("wv", [128, c.KH * c.H], BF16)
    io["wd_in"] = d("wd", [128, c.KH * c.H], BF16)
    io["w1_in"] = d("w1", [c.FF, c.KH * 128], BF16)
    io["w2_in"] = d("w2", [(c.FF // c.FC) * 128, (c.FC // 128) * c.H], BF16)
    io["bq_in"] = d("bq", [128, c.NH])
    io["bk_in"] = d("bk", [128, c.NH])
    io["bd_in"] = d("bd", [1, c.H], BF16)   # SW*(bv@wd + b_dense)
    io["b1_in"] = d("b1", [128, c.FF // 128])
    io["b2_in"] = d("b2", [1, c.H], BF16)   # SW*b2
    io["ones_r_in"] = d("ones_r", [1, c.T], BF16)
    io["onesdr_in"] = d("onesdr", [128, 32], FP8)
    io["cos_in"] = d("cosT", [128, c.T])
    io["sins_in"] = d("sinsT", [128, c.T])
    io["mask_in"] = d("maskT", [c.S, c.T], BF16)   # lm-major key blocks
    io["out_ext"] = nc.dram_tensor("out", [c.T, c.H], F32, kind="ExternalOutput")
    io["groups"] = [[g * c.NG + r for r in range(c.NG)] for g in range(c.B)]

    with tile.TileContext(nc) as tc:
        _body(nc, tc, c, io)
    nc.compile()
    return nc


def _body(nc, tc, c, io):
    x_in, out_ext = io["x_in"], io["out_ext"]
    NT, KH, NH, T, H = c.NT, c.KH, c.NH, c.T, c.H
    WSP, NOS, NG = c.WSP, c.NOS, c.NG
    KH2 = KH // 2
    SKT = c.S // 128
    AXX = mybir.AxisListType.X
    DSC = 1.0 / SW

    # ---------- persistent pools ----------
    const = tc.alloc_tile_pool(name="const", bufs=1)
    ident = const.tile([128, 128], F32, tag="ident", name="ident")
    make_identity(nc, ident[:])
    ones_r = const.tile([1, T], BF16, tag="ones_r", name="ones_r")
    nc.sync.dma_start(ones_r[:], io["ones_r_in"].ap()[:])
    onesdr = const.tile([128, 2, 16], FP8, tag="onesdr", name="onesdr")
    nc.sync.dma_start(onesdr[:].rearrange("p a b -> p (a b)"),
                      io["onesdr_in"].ap()[:])
    epsap = const.tile([128, 1], F32, tag="epsap", name="epsap")
    nc.gpsimd.memset(epsap[:], c.EPS)
    eshift = const.tile([128, 1], F32, tag="eshift", name="eshift")
    nc.gpsimd.memset(eshift[:], -1.5)
    b1_sb = const.tile([128, c.FF // 128], F32, tag="b1", name="b1")
    nc.sync.dma_start(b1_sb[:], io["b1_in"].ap()[:])
    bq_sb = const.tile([128, NH], F32, tag="bq", name="bq")
    nc.sync.dma_start(bq_sb[:], io["bq_in"].ap()[:])
    bk_sb = const.tile([128, NH], F32, tag="bk", name="bk")
    nc.sync.dma_start(bk_sb[:], io["bk_in"].ap()[:])
    cos_sb = const.tile([128, T], F32, tag="cos", name="cos")
    nc.sync.dma_start(cos_sb[:], io["cos_in"].ap()[:])
    sins_sb = const.tile([128, T], F32, tag="sins", name="sins")
    nc.sync.dma_start(sins_sb[:], io["sins_in"].ap()[:])
    stat = tc.alloc_tile_pool(name="stat", bufs=2)
    big = tc.alloc_tile_pool(name="big", bufs=1)
    pp = tc.alloc_tile_pool(name="pp", bufs=1, space="PSUM")
    dram = tc.alloc_tile_pool(name="dram", bufs=1, space="DRAM")

    kT_bnc = [dram.tile([512, T], BF16, tag="kTb", name=f"kTb{cc}", bufs=4)
              for cc in range(4)]
    hid_b = dram.tile([c.T, H], F32, tag="hidb", name="hidb")
    ag_space = maybe_share_collective_output_space("AllGather", io["groups"])
    kT_allc = [dram.tile([NG * 512, T], BF16, tag="kTall", name=f"kTall{cc}",
                         bufs=4, addr_space=ag_space) for cc in range(4)]
    v_bnc = [dram.tile([128, H], FP8, tag="vb", name=f"vb{t}", bufs=NT)
             for t in range(NT)]
    v_alls = [dram.tile([NG * 128, H], FP8, tag="vall", name=f"vall{t}",
                        bufs=NT, addr_space=ag_space) for t in range(NT)]

    NCH = H // 512

    def ln_tile(src, out):
        """LN stats + normalize for one N-layout tile [128, H] -> out."""
        stats = stat.tile([128, NCH, 6], F32, tag="bnst", name="bnst")
        srcr = src[:].rearrange("p (n f) -> p n f", f=512)
        for ch in range(NCH):
            nc.vector.bn_stats(stats[:, ch, :], srcr[:, ch, :])
        mv = stat.tile([128, 2], F32, tag="mv", name="mv")
        nc.vector.bn_aggr(mv[:], stats[:])
        std = stat.tile([128, 1], F32, tag="std", name="std")
        nc.scalar.activation(std[:], mv[:, 1:2], AF.Sqrt, bias=epsap[:],
                             scale=1.0)
        rstd = stat.tile([128, 1], F32, tag="rstd", name="rstd")
        nc.vector.reciprocal(rstd[:], std[:])
        negmr = stat.tile([128, 1], F32, tag="negmr", name="negmr")
        nc.vector.scalar_tensor_tensor(negmr[:], mv[:, 0:1], -1.0, rstd[:],
                                       op0=ALU.mult, op1=ALU.mult)
        nc.scalar.activation(out[:], src[:], AF.Identity,
                             bias=negmr[:], scale=rstd[:])

    def transpose_tile(srcN, dst8, t):
        """[128tok, H] f32 -> cast into fp8 T-layout tile dst8 at column t."""
        for kk in range(KH):
            ps = pp.tile([128, 128], F32, tag="ptr", name="ptr", bufs=2)
            nc.tensor.transpose(ps[:], srcN[:, 128 * kk:128 * (kk + 1)], ident[:])
            nc.vector.tensor_copy(dst8[:, kk:kk + 1, 128 * t:128 * (t + 1)],
                                  ps[:])

    # pb allocated early so the wv prefetch DMA gets a head start
    pb = tc.alloc_tile_pool(name="pb", bufs=1)
    wv_sb = pb.tile([128, KH, H], BF16, tag="wv", name="wv")
    for q4 in range(4):
        nc.sync.dma_start(
            wv_sb[:, 4 * q4:4 * (q4 + 1), :].rearrange("p a b -> p (a b)"),
            io["wv_in"].ap()[:, 4 * q4 * H:4 * (q4 + 1) * H])

    # ---------- Phase A: LN1 + transpose (x streamed) ----------
    xT8 = big.tile([128, KH, T], BF16, tag="TT8", name="xT8", bufs=1)
    for t in range(NT):
        xt = big.tile([128, H], F32, tag="bigH", name=f"x{t}", bufs=4)
        nc.sync.dma_start(xt[:], x_in.ap()[128 * t:128 * (t + 1), :])
        xh = big.tile([128, H], F32, tag="bigH", name=f"xh{t}", bufs=4)
        ln_tile(xt, xh)
        transpose_tile(xh, xT8, t)

    def bail(*pools):
        for p in pools:
            p.release()

    if c.phase_limit <= 1:
        bail(pp, dram, big, stat, const)
        return

    # ---------- Phase B: K -> AG(K); V -> AG(V); Q ----------
    qT = [big.tile([128, T], BF16, tag="qT", name=f"qT{h}", bufs=NH)
          for h in range(NH)]

    def qk_head(h, w_in, b_sb, dst):
        """dst: (dram_ap, row0) or sbuf bf16 tile [128, T]."""
        ps = pp.tile([128, T], F32, tag="mm", name="pqk", bufs=2)
        wt = pb.tile([128, KH, 128], BF16, tag="wqk", name="wqk", bufs=4)
        nc.sync.dma_start(wt[:].rearrange("p a b -> p (a b)"),
                          w_in.ap()[128 * h:128 * (h + 1), :])
        for kk in range(KH):
            nc.tensor.matmul(ps[:], wt[:, kk:kk + 1, :],
                             xT8[:, kk:kk + 1, :],
                             start=(kk == 0), stop=(kk == KH - 1))
        psb = pb.tile([128, T], F32, tag="psb", name="psb", bufs=2)
        nc.scalar.activation(psb[:], ps[:], AF.Identity,
                             bias=b_sb[:, h:h + 1], scale=DSC)
        # partition-swapped biased copy (rotate_half operand), read from PSUM
        psw = pb.tile([128, T], F32, tag="psw", name="psw", bufs=2)
        nc.scalar.activation(psw[0:64, :], ps[64:128, :], AF.Identity,
                             bias=b_sb[64:128, h:h + 1], scale=DSC)
        nc.scalar.activation(psw[64:128, :], ps[0:64, :], AF.Identity,
                             bias=b_sb[0:64, h:h + 1], scale=DSC)
        tmp = pb.tile([128, T], F32, tag="ropetmp", name="ropetmp", bufs=2)
        nc.vector.scalar_tensor_tensor(tmp[:], psw[:], 1.0, sins_sb[:],
                                       op0=ALU.mult, op1=ALU.mult)
        qc = pb.tile([128, T], F32, tag="ropeqc", name="ropeqc", bufs=2)
        nc.vector.scalar_tensor_tensor(qc[:], psb[:], 1.0, cos_sb[:],
                                       op0=ALU.mult, op1=ALU.mult)
        if isinstance(dst, tuple):
            res = pb.tile([128, T], BF16, tag="qkres", name="qkres", bufs=3)
            nc.vector.scalar_tensor_tensor(res[:], qc[:], 1.0, tmp[:],
                                           op0=ALU.mult, op1=ALU.add)
            d_ap, row0 = dst
            nc.sync.dma_start(d_ap[row0:row0 + 128, :], res[:])
        else:
            nc.vector.scalar_tensor_tensor(dst[:], qc[:], 1.0, tmp[:],
                                           op0=ALU.mult, op1=ALU.add)

    for h in range(NH):
        qk_head(h, io["wk_in"], bk_sb, (kT_bnc[h // 4], 128 * (h % 4)))
        if h % 4 == 3:
            nc.gpsimd.collective_compute(
                "AllGather", ALU.bypass, ins=[kT_bnc[h // 4].opt()],
                outs=[kT_allc[h // 4].opt()], replica_groups=io["groups"])

    # V: N-layout DoubleRow; v kept at x32 scale in fp8 (descale in softmax)
    ppv = tc.alloc_tile_pool(name="ppv", bufs=1, space="PSUM")
    for t in range(NT):
        pss = [ppv.tile([128, WSP], F32, tag=f"pvac{o}", name=f"pvac{o}",
                        bufs=1) for o in range(NOS)]
        for kk in range(KH):
            lhs = xT8[:, kk:kk + 1, 128 * t:128 * (t + 1)]
            for osp in range(NOS):
                nc.tensor.matmul(pss[osp][:], lhs,
                                 wv_sb[:, kk:kk + 1,
                                       WSP * osp:WSP * (osp + 1)],
                                 start=(kk == 0), stop=(kk == KH - 1))
        for osp in range(NOS):
            vs = pb.tile([128, WSP], FP8, tag="vslice", name="vslice", bufs=4)
            nc.vector.tensor_copy(vs[:], pss[osp][:])
            nc.sync.dma_start(v_bnc[t][:, WSP * osp:WSP * (osp + 1)], vs[:])
        nc.gpsimd.collective_compute(
            "AllGather", ALU.bypass, ins=[v_bnc[t].opt()],
            outs=[v_alls[t].opt()], replica_groups=io["groups"])
    ppv.release()

    for h in range(NH):
        qk_head(h, io["wq_in"], bq_sb, qT[h])
    pb.release()

    if c.phase_limit <= 2:
        bail(pp, dram, big, stat, const)
        return

    # ---------- Phase C: attention ----------
    # release pp (frees ptr+mm banks) and prefetch dense weights
    pp.release()
    pcd = tc.alloc_tile_pool(name="pcd", bufs=1)
    wd_sb = pcd.tile([128, KH, H], BF16, tag="wd", name="wd")
    for q4 in range(4):
        nc.sync.dma_start(
            wd_sb[:, 4 * q4:4 * (q4 + 1), :].rearrange("p a b -> p (a b)"),
            io["wd_in"].ap()[:, 4 * q4 * H:4 * (q4 + 1) * H])
    pc = tc.alloc_tile_pool(name="pc", bufs=1)
    ppc = tc.alloc_tile_pool(name="ppc", bufs=1, space="PSUM")
    mask_sb = [pcd.tile([128, T], BF16, tag="mask", name=f"mask{m}", bufs=SKT)
               for m in range(SKT)]
    for m in range(SKT):
        nc.sync.dma_start(mask_sb[m][:],
                          io["mask_in"].ap()[128 * m:128 * (m + 1), :])
    ctx8 = pcd.tile([128, NH, T], BF16, tag="ctx8", name="ctx8")
    for h in range(NH):
        kpan = pc.tile([128, NG * T], BF16, tag="kpan", name="kpan", bufs=2)
        cc, hl = h // 4, h % 4
        for rnk in range(NG):
            nc.sync.dma_start(
                kpan[:, rnk * T:(rnk + 1) * T],
                kT_allc[cc][rnk * 512 + 128 * hl:rnk * 512 + 128 * (hl + 1), :])
        vpan8 = pc.tile([128, SKT, 128], FP8, tag="vpan8", name="vpan8",
                        bufs=2)
        for lm in range(NT):
            nc.sync.dma_start(
                vpan8[:, lm * NG:(lm + 1) * NG, :],
                v_alls[lm].rearrange("(r p) cc -> p r cc", p=128)
                [:, :, 128 * h:128 * (h + 1)])
        ems8 = pc.tile([128, SKT, T], FP8, tag="ems8", name="ems8", bufs=1)
        for g in range(SKT):
            lm, rnk = g // NG, g % NG
            ps_s = ppc.tile([128, T], F32, tag="pscore", name="pscore", bufs=2)
            nc.tensor.matmul(
                ps_s[:], kpan[:, rnk * T + 128 * lm:rnk * T + 128 * (lm + 1)],
                qT[h][:], start=True, stop=True)
            e_m = pc.tile([128, T], BF16, tag="eatt", name="eatt", bufs=4)
            # -1.5 shift keeps exp under fp8e4 max (448) for scores < 7.6
            # sigma; cancels in the softmax normalization.
            nc.scalar.activation(e_m[:], ps_s[:], AF.Exp, bias=eshift[:],
                                 scale=c.SCALE)
            nc.vector.scalar_tensor_tensor(ems8[:, g:g + 1, :], e_m[:], 1.0,
                                           mask_sb[g][:], op0=ALU.mult,
                                           op1=ALU.mult)
        ps_ctx = ppc.tile([128, T], F32, tag="pctx", name="pctx", bufs=2)
        ps_sum = ppc.tile([16, T], F32, tag="psml", name="psml", bufs=1)
        for j in range(SKT // 2):
            nc.tensor.matmul(ps_ctx[:], vpan8[:, 2 * j:2 * j + 2, :],
                             ems8[:, 2 * j:2 * j + 2, :],
                             start=(j == 0), stop=(j == SKT // 2 - 1),
                             perf_mode=DR)
        for j in range(SKT // 2):
            nc.tensor.matmul(ps_sum[:], onesdr[:],
                             ems8[:, 2 * j:2 * j + 2, :],
                             start=(j == 0), stop=(j == SKT // 2 - 1),
                             perf_mode=DR)
        rsum = stat.tile([1, T], F32, tag="rsum", name="rsum")
        nc.vector.reciprocal(rsum[:], ps_sum[0:1, :])
        rrep = stat.tile([128, T], F32, tag="rsumrep", name="rsumrep")
        nc.gpsimd.partition_broadcast(rrep[:], rsum[:])
        nc.vector.scalar_tensor_tensor(ctx8[:, h:h + 1, :], ps_ctx[:], DSC,
                                       rrep[:], op0=ALU.mult, op1=ALU.mult)
    ppc.release()
    pc.release()

    if c.phase_limit <= 3:
        bail(pcd, dram, big, stat, const)
        return

    # ---------- Phase D: dense + residual, LN2, transpose ----------
    pd = tc.alloc_tile_pool(name="pd", bufs=1)
    ppd = tc.alloc_tile_pool(name="ppd", bufs=1, space="PSUM")
    bts = []
    for osp in range(NOS):
        bt = pd.tile([1, WSP], BF16, tag="bdsl", name="bdsl", bufs=NOS)
        nc.sync.dma_start(bt[:], io["bd_in"].ap()[:, WSP * osp:WSP * (osp + 1)])
        bts.append(bt)
    for t in range(NT):
        pss = [ppd.tile([128, WSP], F32, tag=f"pdac{o}", name=f"pdac{o}",
                        bufs=1) for o in range(NOS)]
        for kk in range(KH):
            lhs = ctx8[:, kk:kk + 1, 128 * t:128 * (t + 1)]
            for osp in range(NOS):
                nc.tensor.matmul(pss[osp][:], lhs,
                                 wd_sb[:, kk:kk + 1,
                                       WSP * osp:WSP * (osp + 1)],
                                 start=(kk == 0), stop=False)
        for osp in range(NOS):
            nc.tensor.matmul(pss[osp][:], ones_r[:, 0:128], bts[osp][:],
                             start=False, stop=True)
            xs = pd.tile([128, WSP], F32, tag="xsl", name="xsl", bufs=3)
            nc.sync.dma_start(
                xs[:], x_in.ap()[128 * t:128 * (t + 1),
                                 WSP * osp:WSP * (osp + 1)])
            hs = pd.tile([128, WSP], F32, tag="hsl", name="hsl", bufs=3)
            nc.vector.scalar_tensor_tensor(hs[:], pss[osp][:], DSC,
                                           xs[:], op0=ALU.mult, op1=ALU.add)
            nc.sync.dma_start(
                hid_b[128 * t:128 * (t + 1), WSP * osp:WSP * (osp + 1)], hs[:])
    ppd.release()
    pd.release()
    pcd.release()

    # LN2 + transpose (pp re-alloc for transposes + MLP1 psums)
    pp2 = tc.alloc_tile_pool(name="pp2", bufs=1, space="PSUM")
    hT8 = big.tile([128, KH, T], BF16, tag="TT8", name="hT8", bufs=1)
    for t in range(NT):
        ht = big.tile([128, H], F32, tag="bigH", name=f"hid{t}", bufs=4)
        nc.sync.dma_start(ht[:], hid_b[128 * t:128 * (t + 1), :])
        hh = big.tile([128, H], F32, tag="bigH", name=f"hh{t}", bufs=4)
        ln_tile(ht, hh)
        for kk in range(KH):
            ps = pp2.tile([128, 128], F32, tag="ptr2", name="ptr2", bufs=2)
            nc.tensor.transpose(ps[:], hh[:, 128 * kk:128 * (kk + 1)], ident[:])
            nc.vector.tensor_copy(hT8[:, kk:kk + 1, 128 * t:128 * (t + 1)],
                                  ps[:])

    if c.phase_limit <= 4:
        bail(pp2, dram, big, stat, const)
        return

    # ---------- Phase E: fused MLP ----------
    pe = tc.alloc_tile_pool(name="pe", bufs=1)
    ppe2 = tc.alloc_tile_pool(name="ppe2", bufs=1, space="PSUM")
    NFC = c.FF // c.FC
    FCT = c.FC // 128
    out_t = [big.tile([128, H], F32, tag="bigH", name=f"out{t}", bufs=4)
             for t in range(NT)]
    for f in range(NFC):
        g8 = pe.tile([128, FCT, T], BF16, tag="g8", name="g8", bufs=2)
        for mm in range(FCT):
            fglob = f * FCT + mm
            w1t = pe.tile([128, KH, 128], BF16, tag="w1pan", name="w1pan",
                          bufs=4)
            nc.sync.dma_start(w1t[:].rearrange("p a b -> p (a b)"),
                              io["w1_in"].ap()[128 * fglob:128 * (fglob + 1), :])
            ps = pp2.tile([128, T], F32, tag="mm1", name="pm1", bufs=2)
            for kk in range(KH):
                nc.tensor.matmul(ps[:], w1t[:, kk:kk + 1, :],
                                 hT8[:, kk:kk + 1, :],
                                 start=(kk == 0), stop=(kk == KH - 1))
            nc.scalar.activation(g8[:, mm:mm + 1, :], ps[:], AF.Gelu,
                                 bias=b1_sb[:, fglob:fglob + 1], scale=DSC)
        w2t = pe.tile([128, FCT, H], BF16, tag="w2pan", name="w2pan", bufs=2)
        for q2 in range(2):
            hfc = FCT // 2
            nc.sync.dma_start(
                w2t[:, hfc * q2:hfc * (q2 + 1), :].rearrange("p a b -> p (a b)"),
                io["w2_in"].ap()[128 * f:128 * (f + 1),
                                 hfc * q2 * H:hfc * (q2 + 1) * H])
        if f == 0:
            b2s = []
            for osp in range(NOS):
                bt = pe.tile([1, WSP], BF16, tag="b2sl", name="b2sl", bufs=NOS)
                nc.sync.dma_start(
                    bt[:], io["b2_in"].ap()[:, WSP * osp:WSP * (osp + 1)])
                b2s.append(bt)
        for t in range(NT):
            pss = [ppe2.tile([128, WSP], F32, tag=f"pmac{o}", name=f"pmac{o}",
                             bufs=1) for o in range(NOS)]
            for kf in range(FCT):
                lhs = g8[:, kf:kf + 1, 128 * t:128 * (t + 1)]
                for osp in range(NOS):
                    nc.tensor.matmul(pss[osp][:], lhs,
                                     w2t[:, kf:kf + 1,
                                         WSP * osp:WSP * (osp + 1)],
                                     start=(kf == 0),
                                     stop=(kf == FCT - 1 and f != 0))
            for osp in range(NOS):
                osl = out_t[t][:, WSP * osp:WSP * (osp + 1)]
                if f == 0:
                    nc.tensor.matmul(pss[osp][:], ones_r[:, 0:128], b2s[osp][:],
                                     start=False, stop=True)
                    hsl = pe.tile([128, WSP], F32, tag="hres", name="hres",
                                  bufs=3)
                    nc.sync.dma_start(
                        hsl[:], hid_b[128 * t:128 * (t + 1),
                                      WSP * osp:WSP * (osp + 1)])
                    nc.vector.scalar_tensor_tensor(osl, pss[osp][:], DSC,
                                                   hsl[:], op0=ALU.mult,
                                                   op1=ALU.add)
                else:
                    nc.vector.scalar_tensor_tensor(osl, pss[osp][:], DSC, osl,
                                                   op0=ALU.mult, op1=ALU.add)
    ppe2.release()
    pe.release()

    # ---------- Phase F: output ----------
    for t in range(NT):
        nc.sync.dma_start(out_ext.ap()[128 * t:128 * (t + 1), :], out_t[t][:])

    for p in (pp2, dram, big, stat, const):
        p.release()


# ---------------- host side ----------------

def prepare_in_maps(c: Cfg, inputs):
    f32 = np.float32
    hs = np.asarray(inputs["hidden_states"], f32)
    ln1_g = np.asarray(inputs["ln1_g"], f32)
    ln1_b = np.asarray(inputs["ln1_b"], f32)
    w_qkv = np.asarray(inputs["w_qkv"], f32)
    b_qkv = np.asarray(inputs["b_qkv"], f32)
    w_dense = np.asarray(inputs["w_dense"], f32)
    b_dense = np.asarray(inputs["b_dense"], f32)
    ln2_g = np.asarray(inputs["ln2_g"], f32)
    ln2_b = np.asarray(inputs["ln2_b"], f32)
    w1 = np.asarray(inputs["w1"], f32)
    b1 = np.asarray(inputs["b1"], f32)
    w2 = np.asarray(inputs["w2"], f32)
    b2 = np.asarray(inputs["b2"], f32)

    H, NH, HD, FF, KH = c.H, c.NH, c.HD, c.FF, c.KH
    NFC, FCT = FF // c.FC, c.FC // 128
    cols = np.concatenate([np.arange(h * 3 * HD, h * 3 * HD + HD)
                           for h in range(NH)])
    wg = ln1_g[:, None] * w_qkv
    wq_f, wk_f, wv_f = wg[:, cols], wg[:, cols + HD], wg[:, cols + 2 * HD]
    bfull = ln1_b @ w_qkv + b_qkv
    bq_f, bk_f, bv_f = bfull[cols], bfull[cols + HD], bfull[cols + 2 * HD]
    bd_f = bv_f @ w_dense + b_dense          # v-bias folded through attention
    w1_f = ln2_g[:, None] * w1
    b1_f = ln2_b @ w1 + b1

    # head-major rows [NH*128, KH*128]: block h = weights for head h
    hmaj = lambda w, nb: np.ascontiguousarray(
        w.reshape(KH, 128, nb, 128).transpose(2, 1, 0, 3)
        .reshape(nb * 128, KH * 128).astype(BF))
    # [128, KH*H]: row p, col kk*H + cc  =  w[kk*128+p, cc]
    parr = lambda w: np.ascontiguousarray(
        w.reshape(KH, 128, H).transpose(1, 0, 2)
        .reshape(128, KH * H).astype(BF))
    wqh = hmaj(wq_f, NH)
    wkh = hmaj(wk_f, NH)
    wvh = parr(wv_f)
    wdh = parr(w_dense)
    w1h = hmaj(w1_f, FF // 128)
    w2h = np.ascontiguousarray(
        w2.reshape(NFC, FCT, 128, H).transpose(0, 2, 1, 3)
        .reshape(NFC * 128, FCT * H).astype(BF))

    inv = 1.0 / (10000.0 ** (np.arange(0, HD, 2, dtype=f32) / HD))
    pos = np.arange(c.S, dtype=f32)
    frq = np.einsum('i,j->ij', pos, inv)
    emb = np.concatenate([frq, frq], axis=-1)
    cos_full = np.cos(emb).T.astype(f32)
    sin_full = np.sin(emb).T.astype(f32)
    sins_full = sin_full.copy()
    sins_full[:HD // 2] *= -1.0

    bf = lambda a: np.ascontiguousarray(a.astype(BF))
    in_maps = []
    for i in range(c.W):
        b, g = i // c.NG, i % c.NG
        t0 = g * c.T
        qpos = np.arange(t0, t0 + c.T)
        # lm-major mask: slot gp = lm*NG + rnk covers key block rnk*NT + lm
        mrows = []
        for gp in range(c.S // 128):
            lm, rnk = gp // c.NG, gp % c.NG
            kb = rnk * c.NT + lm
            kpos = np.arange(kb * 128, (kb + 1) * 128)
            mrows.append((kpos[:, None] <= qpos[None, :]).astype(BF))
        mask = np.concatenate(mrows, axis=0)
        in_maps.append({
            "x": np.ascontiguousarray(hs[b, t0:t0 + c.T, :]),
            "wq": wqh, "wk": wkh, "wv": wvh, "wd": wdh, "w1": w1h, "w2": w2h,
            "bq": np.ascontiguousarray(bq_f.reshape(NH, 128).T.astype(f32)),
            "bk": np.ascontiguousarray(bk_f.reshape(NH, 128).T.astype(f32)),
            "bd": bf(SW * bd_f.reshape(1, H)),
            "b1": np.ascontiguousarray(b1_f.reshape(FF // 128, 128).T
                                       .astype(f32)),
            "b2": bf(SW * b2.reshape(1, H)),
            "ones_r": np.ones((1, c.T), BF),
            "onesdr": np.ones((128, 32), E4),
            "cosT": np.ascontiguousarray(cos_full[:, t0:t0 + c.T]),
            "sinsT": np.ascontiguousarray(sins_full[:, t0:t0 + c.T]),
            "maskT": np.ascontiguousarray(mask),
        })
    return in_maps


def assemble_output(c: Cfg, results):
    out = np.empty((c.B, c.S, c.H), np.float32)
    for i in range(c.W):
        b, g = i // c.NG, i % c.NG
        out[b, g * c.T:(g + 1) * c.T, :] = results[i]["out"]
    return out


def run(nc, c: Cfg, inputs, trace=False, **kw):
    in_maps = prepare_in_maps(c, inputs)
    last = None
    for attempt in range(3):
        try:
            res = bass_utils.run_bass_kernel_spmd(
                nc, in_maps, core_ids=list(range(c.W)), trace=trace, **kw)
            return assemble_output(c, res.results), res
        except Exception as e:
            last = e
            print(f"run attempt {attempt} failed: {type(e).__name__}: {e}",
                  file=sys.stderr)
    raise last


# ======================= harness entry point =======================

_CACHE = {}


def kernel(**inputs):
    """Full-input entry: shard, compile (cached), run on 8 cores, gather."""
    c = Cfg()
    if "nc" not in _CACHE:
        _CACHE["nc"] = build(c)
    out, _ = run(_CACHE["nc"], c, inputs, trace=False)
    return out


# revision 42
# speedup vs baseline: 1.0272x; 1.0112x over previous
"""Fused GPT transformer layer on 8 trn2 cores — token-parallel + KV AllGather.

Sharding: core i owns 512 contiguous tokens (cores 0-3 batch 0, 4-7 batch 1).
Per core: LN1 -> QKV (+RoPE) local; AllGather K^T (bf16) and V (fp8) within
4-core group; masked full-key attention (softmax without max-subtraction);
dense+residual, LN2, fused chunked MLP all local. Host gathers outputs.

v4: all big GEMMs (QKV, V, dense, MLP1, MLP2) run fp8e4 DoubleRow (K=256 per
matmul, 2x FLOP/instr at the same 220ns/MM issue rate). Weights pre-scaled
x32 into fp8 on host; descale 1/32 folded into PSUM-evacuation activations.
Attention scores stay bf16; probs+V are fp8 so context & softmax-denominator
matmuls are DoubleRow too. Q/K bias folded into scalar.activation (T-layout
per-partition bias). Order K -> AG(K) -> V -> AG(V) -> Q -> attention so
local compute hides both collectives. Hidden state kept in SBUF (no DRAM
bounce). Weights pre-arranged on host for contiguous per-partition DMA.

Layouts: "N" = [token-partition, feature-free]; "T" = [feature-part, token-free].
fp8 3D tiles [128, k-block, free] feed DoubleRow pairs [:, 2k:2k+2, :].
"""
import sys
if '/opt/trn_rl_repo' not in sys.path:
    sys.path.insert(0, '/opt/trn_rl_repo')

from dataclasses import dataclass

import numpy as np
import ml_dtypes

import concourse.bass as bass
import concourse.bacc as bacc
import concourse.tile as tile
import concourse.mybir as mybir
from concourse import bass_utils
from concourse.masks import make_identity
from concourse.replica_groups import maybe_share_collective_output_space

F32 = mybir.dt.float32
BF16 = mybir.dt.bfloat16
FP8 = mybir.dt.float8e4
AF = mybir.ActivationFunctionType
ALU = mybir.AluOpType
DR = mybir.MatmulPerfMode.DoubleRow
BF = ml_dtypes.bfloat16
E4 = ml_dtypes.float8_e4m3fn
SW = 1.0    # bf16 weights: no pre-scale


@dataclass
class Cfg:
    B: int = 2
    S: int = 2048
    H: int = 2048
    NH: int = 16
    FF: int = 8192
    W: int = 8           # total cores
    FC: int = 1024       # FF chunk for fused MLP
    WSP: int = 512       # weight panel span (moving free dim for N-layout mms)
    EPS: float = 1e-5
    phase_limit: int = 99   # 1=A, 2=B(+AG), 3=C, 4=D, 5=E

    @property
    def HD(self):
        return self.H // self.NH

    @property
    def NG(self):
        return self.W // self.B

    @property
    def T(self):
        return self.S // self.NG

    @property
    def NT(self):
        return self.T // 128

    @property
    def KH(self):
        return self.H // 128

    @property
    def NOS(self):
        return self.H // self.WSP

    @property
    def SCALE(self):
        return 1.0 / float(np.sqrt(self.HD))


def build(cfg: Cfg):
    c = cfg
    assert c.HD == 128 and c.T % 128 == 0 and c.H % c.WSP == 0
    assert c.FF % c.FC == 0 and c.FC % 128 == 0

    nc = bacc.Bacc("TRN2", target_bir_lowering=False, debug=False,
                   num_devices=c.W)
    d = lambda name, shape, dt=F32: nc.dram_tensor(name, shape, dt,
                                                   kind="ExternalInput")
    io = {}
    io["x_in"] = d("x", [c.T, c.H])
    io["wq_in"] = d("wq", [c.NH * 128, c.KH * 128], BF16)
    io["wk_in"] = d("wk", [c.NH * 128, c.KH * 128], BF16)
    io["wv_in"] = d("wv", [128, c.KH * c.H], BF16)
    io["wd_in"] = d("wd", [128, c.KH * c.H], BF16)
    io["w1_in"] = d("w1", [c.FF, c.KH * 128], BF16)
    io["w2_in"] = d("w2", [(c.FF // c.FC) * 128, (c.FC // 128) * c.H], BF16)
    io["bq_in"] = d("bq", [128, c.NH])
    io["bk_in"] = d("bk", [128, c.NH])
    io["bd_in"] = d("bd", [1, c.H], BF16)   # SW*(bv@wd + b_dense)
    io["b1_in"] = d("b1", [128, c.FF // 128])
    io["b2_in"] = d("b2", [1, c.H], BF16)   # SW*b2
    io["ones_r_in"] = d("ones_r", [1, c.T], BF16)
    io["onesdr_in"] = d("onesdr", [128, 32], FP8)
    io["cos_in"] = d("cosT", [128, c.T])
    io["sins_in"] = d("sinsT", [128, c.T])
    io["mask_in"] = d("maskT", [c.S, c.T], BF16)   # lm-major key blocks
    io["out_ext"] = nc.dram_tensor("out", [c.T, c.H], F32, kind="ExternalOutput")
    io["groups"] = [[g * c.NG + r for r in range(c.NG)] for g in range(c.B)]

    with tile.TileContext(nc) as tc:
        _body(nc, tc, c, io)
    nc.compile()
    return nc


def _body(nc, tc, c, io):
    x_in, out_ext = io["x_in"], io["out_ext"]
    NT, KH, NH, T, H = c.NT, c.KH, c.NH, c.T, c.H
    WSP, NOS, NG = c.WSP, c.NOS, c.NG
    KH2 = KH // 2
    SKT = c.S // 128
    AXX = mybir.AxisListType.X
    DSC = 1.0 / SW

    # ---------- persistent pools ----------
    const = tc.alloc_tile_pool(name="const", bufs=1)
    ident = const.tile([128, 128], F32, tag="ident", name="ident")
    make_identity(nc, ident[:])
    ones_r = const.tile([1, T], BF16, tag="ones_r", name="ones_r")
    nc.sync.dma_start(ones_r[:], io["ones_r_in"].ap()[:])
    onesdr = const.tile([128, 2, 16], FP8, tag="onesdr", name="onesdr")
    nc.sync.dma_start(onesdr[:].rearrange("p a b -> p (a b)"),
                      io["onesdr_in"].ap()[:])
    epsap = const.tile([128, 1], F32, tag="epsap", name="epsap")
    nc.gpsimd.memset(epsap[:], c.EPS)
    eshift = const.tile([128, 1], F32, tag="eshift", name="eshift")
    nc.gpsimd.memset(eshift[:], -1.5)
    b1_sb = const.tile([128, c.FF // 128], F32, tag="b1", name="b1")
    nc.sync.dma_start(b1_sb[:], io["b1_in"].ap()[:])
    bq_sb = const.tile([128, NH], F32, tag="bq", name="bq")
    nc.sync.dma_start(bq_sb[:], io["bq_in"].ap()[:])
    bk_sb = const.tile([128, NH], F32, tag="bk", name="bk")
    nc.sync.dma_start(bk_sb[:], io["bk_in"].ap()[:])
    cos_sb = const.tile([128, T], F32, tag="cos", name="cos")
    nc.sync.dma_start(cos_sb[:], io["cos_in"].ap()[:])
    sins_sb = const.tile([128, T], F32, tag="sins", name="sins")
    nc.sync.dma_start(sins_sb[:], io["sins_in"].ap()[:])
    stat = tc.alloc_tile_pool(name="stat", bufs=2)
    big = tc.alloc_tile_pool(name="big", bufs=1)
    pp = tc.alloc_tile_pool(name="pp", bufs=1, space="PSUM")
    dram = tc.alloc_tile_pool(name="dram", bufs=1, space="DRAM")

    kT_bnc = [dram.tile([512, T], BF16, tag="kTb", name=f"kTb{cc}", bufs=4)
              for cc in range(4)]
    hid_b = dram.tile([c.T, H], F32, tag="hidb", name="hidb")
    ag_space = maybe_share_collective_output_space("AllGather", io["groups"])
    kT_allc = [dram.tile([NG * 512, T], BF16, tag="kTall", name=f"kTall{cc}",
                         bufs=4, addr_space=ag_space) for cc in range(4)]
    v_bnc = [dram.tile([128, H], FP8, tag="vb", name=f"vb{t}", bufs=NT)
             for t in range(NT)]
    v_alls = [dram.tile([NG * 128, H], FP8, tag="vall", name=f"vall{t}",
                        bufs=NT, addr_space=ag_space) for t in range(NT)]

    NCH = H // 512

    def ln_tile(src, out):
        """LN stats + normalize for one N-layout tile [128, H] -> out."""
        stats = stat.tile([128, NCH, 6], F32, tag="bnst", name="bnst")
        srcr = src[:].rearrange("p (n f) -> p n f", f=512)
        for ch in range(NCH):
            nc.vector.bn_stats(stats[:, ch, :], srcr[:, ch, :])
        mv = stat.tile([128, 2], F32, tag="mv", name="mv")
        nc.vector.bn_aggr(mv[:], stats[:])
        std = stat.tile([128, 1], F32, tag="std", name="std")
        nc.scalar.activation(std[:], mv[:, 1:2], AF.Sqrt, bias=epsap[:],
                             scale=1.0)
        rstd = stat.tile([128, 1], F32, tag="rstd", name="rstd")
        nc.vector.reciprocal(rstd[:], std[:])
        negmr = stat.tile([128, 1], F32, tag="negmr", name="negmr")
        nc.vector.scalar_tensor_tensor(negmr[:], mv[:, 0:1], -1.0, rstd[:],
                                       op0=ALU.mult, op1=ALU.mult)
        nc.scalar.activation(out[:], src[:], AF.Identity,
                             bias=negmr[:], scale=rstd[:])

    def transpose_tile(srcN, dst8, t):
        """[128tok, H] f32 -> cast into fp8 T-layout tile dst8 at column t."""
        for kk in range(KH):
            ps = pp.tile([128, 128], F32, tag="ptr", name="ptr", bufs=2)
            nc.tensor.transpose(ps[:], srcN[:, 128 * kk:128 * (kk + 1)], ident[:])
            nc.vector.tensor_copy(dst8[:, kk:kk + 1, 128 * t:128 * (t + 1)],
                                  ps[:])

    # pb allocated early so the wv prefetch DMA gets a head start
    pb = tc.alloc_tile_pool(name="pb", bufs=1)
    wv_sb = pb.tile([128, KH, H], BF16, tag="wv", name="wv")
    for q4 in range(4):
        nc.sync.dma_start(
            wv_sb[:, 4 * q4:4 * (q4 + 1), :].rearrange("p a b -> p (a b)"),
            io["wv_in"].ap()[:, 4 * q4 * H:4 * (q4 + 1) * H])

    # ---------- Phase A: LN1 + transpose (x streamed) ----------
    xT8 = big.tile([128, KH, T], BF16, tag="TT8", name="xT8", bufs=1)
    for t in range(NT):
        xt = big.tile([128, H], F32, tag="bigH", name=f"x{t}", bufs=4)
        nc.sync.dma_start(xt[:], x_in.ap()[128 * t:128 * (t + 1), :])
        xh = big.tile([128, H], F32, tag="bigH", name=f"xh{t}", bufs=4)
        ln_tile(xt, xh)
        transpose_tile(xh, xT8, t)

    def bail(*pools):
        for p in pools:
            p.release()

    if c.phase_limit <= 1:
        bail(pp, dram, big, stat, const)
        return

    # ---------- Phase B: K -> AG(K); V -> AG(V); Q ----------
    qT = [big.tile([128, T], BF16, tag="qT", name=f"qT{h}", bufs=NH)
          for h in range(NH)]

    def qk_head(h, w_in, b_sb, dst):
        """dst: (dram_ap, row0) or sbuf bf16 tile [128, T]."""
        ps = pp.tile([128, T], F32, tag="mm", name="pqk", bufs=2)
        wt = pb.tile([128, KH, 128], BF16, tag="wqk", name="wqk", bufs=4)
        nc.sync.dma_start(wt[:].rearrange("p a b -> p (a b)"),
                          w_in.ap()[128 * h:128 * (h + 1), :])
        for kk in range(KH):
            nc.tensor.matmul(ps[:], wt[:, kk:kk + 1, :],
                             xT8[:, kk:kk + 1, :],
                             start=(kk == 0), stop=(kk == KH - 1))
        psb = pb.tile([128, T], F32, tag="psb", name="psb", bufs=2)
        nc.scalar.activation(psb[:], ps[:], AF.Identity,
                             bias=b_sb[:, h:h + 1], scale=DSC)
        # partition-swapped biased copy (rotate_half operand), read from PSUM
        psw = pb.tile([128, T], F32, tag="psw", name="psw", bufs=2)
        nc.scalar.activation(psw[0:64, :], ps[64:128, :], AF.Identity,
                             bias=b_sb[64:128, h:h + 1], scale=DSC)
        nc.scalar.activation(psw[64:128, :], ps[0:64, :], AF.Identity,
                             bias=b_sb[0:64, h:h + 1], scale=DSC)
        tmp = pb.tile([128, T], F32, tag="ropetmp", name="ropetmp", bufs=2)
        nc.vector.scalar_tensor_tensor(tmp[:], psw[:], 1.0, sins_sb[:],
                                       op0=ALU.mult, op1=ALU.mult)
        qc = pb.tile([128, T], F32, tag="ropeqc", name="ropeqc", bufs=2)
        nc.vector.scalar_tensor_tensor(qc[:], psb[:], 1.0, cos_sb[:],
                                       op0=ALU.mult, op1=ALU.mult)
        if isinstance(dst, tuple):
            res = pb.tile([128, T], BF16, tag="qkres", name="qkres", bufs=3)
            nc.vector.scalar_tensor_tensor(res[:], qc[:], 1.0, tmp[:],
                                           op0=ALU.mult, op1=ALU.add)
            d_ap, row0 = dst
            nc.sync.dma_start(d_ap[row0:row0 + 128, :], res[:])
        else:
            nc.vector.scalar_tensor_tensor(dst[:], qc[:], 1.0, tmp[:],
                                           op0=ALU.mult, op1=ALU.add)

    for h in range(NH):
        qk_head(h, io["wk_in"], bk_sb, (kT_bnc[h // 4], 128 * (h % 4)))
        if h % 4 == 3:
            nc.gpsimd.collective_compute(
                "AllGather", ALU.bypass, ins=[kT_bnc[h // 4].opt()],
                outs=[kT_allc[h // 4].opt()], replica_groups=io["groups"])

    # V: N-layout DoubleRow; v kept at x32 scale in fp8 (descale in softmax)
    ppv = tc.alloc_tile_pool(name="ppv", bufs=1, space="PSUM")
    for t in range(NT):
        pss = [ppv.tile([128, WSP], F32, tag=f"pvac{o}", name=f"pvac{o}",
                        bufs=1) for o in range(NOS)]
        for kk in range(KH):
            lhs = xT8[:, kk:kk + 1, 128 * t:128 * (t + 1)]
            for osp in range(NOS):
                nc.tensor.matmul(pss[osp][:], lhs,
                                 wv_sb[:, kk:kk + 1,
                                       WSP * osp:WSP * (osp + 1)],
                                 start=(kk == 0), stop=(kk == KH - 1))
        for osp in range(NOS):
            vs = pb.tile([128, WSP], FP8, tag="vslice", name="vslice", bufs=4)
            nc.vector.tensor_copy(vs[:], pss[osp][:])
            nc.sync.dma_start(v_bnc[t][:, WSP * osp:WSP * (osp + 1)], vs[:])
        nc.gpsimd.collective_compute(
            "AllGather", ALU.bypass, ins=[v_bnc[t].opt()],
            outs=[v_alls[t].opt()], replica_groups=io["groups"])
    ppv.release()

    for h in range(NH):
        qk_head(h, io["wq_in"], bq_sb, qT[h])
    pb.release()

    if c.phase_limit <= 2:
        bail(pp, dram, big, stat, const)
        return

    # ---------- Phase C: attention ----------
    # release pp (frees ptr+mm banks) and prefetch dense weights
    pp.release()
    pcd = tc.alloc_tile_pool(name="pcd", bufs=1)
    wd_sb = pcd.tile([128, KH, H], BF16, tag="wd", name="wd")
    for q4 in range(4):
        nc.sync.dma_start(
            wd_sb[:, 4 * q4:4 * (q4 + 1), :].rearrange("p a b -> p (a b)"),
            io["wd_in"].ap()[:, 4 * q4 * H:4 * (q4 + 1) * H])
    pc = tc.alloc_tile_pool(name="pc", bufs=1)
    ppc = tc.alloc_tile_pool(name="ppc", bufs=1, space="PSUM")
    mask_sb = [pcd.tile([128, T], BF16, tag="mask", name=f"mask{m}", bufs=SKT)
               for m in range(SKT)]
    for m in range(SKT):
        nc.sync.dma_start(mask_sb[m][:],
                          io["mask_in"].ap()[128 * m:128 * (m + 1), :])
    ctx8 = pcd.tile([128, NH, T], BF16, tag="ctx8", name="ctx8")
    for h in range(NH):
        kpan = pc.tile([128, NG * T], BF16, tag="kpan", name="kpan", bufs=2)
        cc, hl = h // 4, h % 4
        for rnk in range(NG):
            nc.sync.dma_start(
                kpan[:, rnk * T:(rnk + 1) * T],
                kT_allc[cc][rnk * 512 + 128 * hl:rnk * 512 + 128 * (hl + 1), :])
        vpan8 = pc.tile([128, SKT, 128], FP8, tag="vpan8", name="vpan8",
                        bufs=2)
        for lm in range(NT):
            nc.sync.dma_start(
                vpan8[:, lm * NG:(lm + 1) * NG, :],
                v_alls[lm].rearrange("(r p) cc -> p r cc", p=128)
                [:, :, 128 * h:128 * (h + 1)])
        ems8 = pc.tile([128, SKT, T], FP8, tag="ems8", name="ems8", bufs=2)
        for g in range(SKT):
            lm, rnk = g // NG, g % NG
            ps_s = ppc.tile([128, T], F32, tag="pscore", name="pscore", bufs=2)
            nc.tensor.matmul(
                ps_s[:], kpan[:, rnk * T + 128 * lm:rnk * T + 128 * (lm + 1)],
                qT[h][:], start=True, stop=True)
            e_m = pc.tile([128, T], BF16, tag="eatt", name="eatt", bufs=4)
            # -1.5 shift keeps exp under fp8e4 max (448) for scores < 7.6
            # sigma; cancels in the softmax normalization.
            nc.scalar.activation(e_m[:], ps_s[:], AF.Exp, bias=eshift[:],
                                 scale=c.SCALE)
            nc.vector.scalar_tensor_tensor(ems8[:, g:g + 1, :], e_m[:], 1.0,
                                           mask_sb[g][:], op0=ALU.mult,
                                           op1=ALU.mult)
        ps_ctx = ppc.tile([128, T], F32, tag="pctx", name="pctx", bufs=2)
        ps_sum = ppc.tile([16, T], F32, tag="psml", name="psml", bufs=2)
        for j in range(SKT // 2):
            nc.tensor.matmul(ps_ctx[:], vpan8[:, 2 * j:2 * j + 2, :],
                             ems8[:, 2 * j:2 * j + 2, :],
                             start=(j == 0), stop=(j == SKT // 2 - 1),
                             perf_mode=DR)
        for j in range(SKT // 2):
            nc.tensor.matmul(ps_sum[:], onesdr[:],
                             ems8[:, 2 * j:2 * j + 2, :],
                             start=(j == 0), stop=(j == SKT // 2 - 1),
                             perf_mode=DR)
        rsum = stat.tile([1, T], F32, tag="rsum", name="rsum")
        nc.vector.reciprocal(rsum[:], ps_sum[0:1, :])
        rrep = stat.tile([128, T], F32, tag="rsumrep", name="rsumrep")
        nc.gpsimd.partition_broadcast(rrep[:], rsum[:])
        nc.vector.scalar_tensor_tensor(ctx8[:, h:h + 1, :], ps_ctx[:], DSC,
                                       rrep[:], op0=ALU.mult, op1=ALU.mult)
    ppc.release()
    pc.release()

    if c.phase_limit <= 3:
        bail(pcd, dram, big, stat, const)
        return

    # ---------- Phase D: dense + residual, LN2, transpose ----------
    pd = tc.alloc_tile_pool(name="pd", bufs=1)
    ppd = tc.alloc_tile_pool(name="ppd", bufs=1, space="PSUM")
    bts = []
    for osp in range(NOS):
        bt = pd.tile([1, WSP], BF16, tag="bdsl", name="bdsl", bufs=NOS)
        nc.sync.dma_start(bt[:], io["bd_in"].ap()[:, WSP * osp:WSP * (osp + 1)])
        bts.append(bt)
    for t in range(NT):
        pss = [ppd.tile([128, WSP], F32, tag=f"pdac{o}", name=f"pdac{o}",
                        bufs=1) for o in range(NOS)]
        for kk in range(KH):
            lhs = ctx8[:, kk:kk + 1, 128 * t:128 * (t + 1)]
            for osp in range(NOS):
                nc.tensor.matmul(pss[osp][:], lhs,
                                 wd_sb[:, kk:kk + 1,
                                       WSP * osp:WSP * (osp + 1)],
                                 start=(kk == 0), stop=False)
        for osp in range(NOS):
            nc.tensor.matmul(pss[osp][:], ones_r[:, 0:128], bts[osp][:],
                             start=False, stop=True)
            xs = pd.tile([128, WSP], F32, tag="xsl", name="xsl", bufs=3)
            nc.sync.dma_start(
                xs[:], x_in.ap()[128 * t:128 * (t + 1),
                                 WSP * osp:WSP * (osp + 1)])
            hs = pd.tile([128, WSP], F32, tag="hsl", name="hsl", bufs=3)
            nc.vector.scalar_tensor_tensor(hs[:], pss[osp][:], DSC,
                                           xs[:], op0=ALU.mult, op1=ALU.add)
            nc.sync.dma_start(
                hid_b[128 * t:128 * (t + 1), WSP * osp:WSP * (osp + 1)], hs[:])
    ppd.release()
    pd.release()
    pcd.release()

    # LN2 + transpose (pp re-alloc for transposes + MLP1 psums)
    pp2 = tc.alloc_tile_pool(name="pp2", bufs=1, space="PSUM")
    hT8 = big.tile([128, KH, T], BF16, tag="TT8", name="hT8", bufs=1)
    for t in range(NT):
        ht = big.tile([128, H], F32, tag="bigH", name=f"hid{t}", bufs=4)
        nc.sync.dma_start(ht[:], hid_b[128 * t:128 * (t + 1), :])
        hh = big.tile([128, H], F32, tag="bigH", name=f"hh{t}", bufs=4)
        ln_tile(ht, hh)
        for kk in range(KH):
            ps = pp2.tile([128, 128], F32, tag="ptr2", name="ptr2", bufs=2)
            nc.tensor.transpose(ps[:], hh[:, 128 * kk:128 * (kk + 1)], ident[:])
            nc.vector.tensor_copy(hT8[:, kk:kk + 1, 128 * t:128 * (t + 1)],
                                  ps[:])

    if c.phase_limit <= 4:
        bail(pp2, dram, big, stat, const)
        return

    # ---------- Phase E: fused MLP ----------
    pe = tc.alloc_tile_pool(name="pe", bufs=1)
    ppe2 = tc.alloc_tile_pool(name="ppe2", bufs=1, space="PSUM")
    NFC = c.FF // c.FC
    FCT = c.FC // 128
    out_t = [big.tile([128, H], F32, tag="bigH", name=f"out{t}", bufs=4)
             for t in range(NT)]
    for f in range(NFC):
        g8 = pe.tile([128, FCT, T], BF16, tag="g8", name="g8", bufs=2)
        for mm in range(FCT):
            fglob = f * FCT + mm
            w1t = pe.tile([128, KH, 128], BF16, tag="w1pan", name="w1pan",
                          bufs=4)
            nc.sync.dma_start(w1t[:].rearrange("p a b -> p (a b)"),
                              io["w1_in"].ap()[128 * fglob:128 * (fglob + 1), :])
            ps = pp2.tile([128, T], F32, tag="mm1", name="pm1", bufs=2)
            for kk in range(KH):
                nc.tensor.matmul(ps[:], w1t[:, kk:kk + 1, :],
                                 hT8[:, kk:kk + 1, :],
                                 start=(kk == 0), stop=(kk == KH - 1))
            nc.scalar.activation(g8[:, mm:mm + 1, :], ps[:], AF.Gelu,
                                 bias=b1_sb[:, fglob:fglob + 1], scale=DSC)
        w2t = pe.tile([128, FCT, H], BF16, tag="w2pan", name="w2pan", bufs=2)
        for q2 in range(2):
            hfc = FCT // 2
            nc.sync.dma_start(
                w2t[:, hfc * q2:hfc * (q2 + 1), :].rearrange("p a b -> p (a b)"),
                io["w2_in"].ap()[128 * f:128 * (f + 1),
                                 hfc * q2 * H:hfc * (q2 + 1) * H])
        if f == 0:
            b2s = []
            for osp in range(NOS):
                bt = pe.tile([1, WSP], BF16, tag="b2sl", name="b2sl", bufs=NOS)
                nc.sync.dma_start(
                    bt[:], io["b2_in"].ap()[:, WSP * osp:WSP * (osp + 1)])
                b2s.append(bt)
        for t in range(NT):
            pss = [ppe2.tile([128, WSP], F32, tag=f"pmac{o}", name=f"pmac{o}",
                             bufs=1) for o in range(NOS)]
            for kf in range(FCT):
                lhs = g8[:, kf:kf + 1, 128 * t:128 * (t + 1)]
                for osp in range(NOS):
                    nc.tensor.matmul(pss[osp][:], lhs,
                                     w2t[:, kf:kf + 1,
                                         WSP * osp:WSP * (osp + 1)],
                                     start=(kf == 0),
                                     stop=(kf == FCT - 1 and f != 0))
            for osp in range(NOS):
                osl = out_t[t][:, WSP * osp:WSP * (osp + 1)]
                if f == 0:
                    nc.tensor.matmul(pss[osp][:], ones_r[:, 0:128], b2s[osp][:],
                                     start=False, stop=True)
                    hsl = pe.tile([128, WSP], F32, tag="hres", name="hres",
                                  bufs=3)
                    nc.sync.dma_start(
                        hsl[:], hid_b[128 * t:128 * (t + 1),
                                      WSP * osp:WSP * (osp + 1)])
                    nc.vector.scalar_tensor_tensor(osl, pss[osp][:], DSC,
                                                   hsl[:], op0=ALU.mult,
                                                   op1=ALU.add)
                else:
                    nc.vector.scalar_tensor_tensor(osl, pss[osp][:], DSC, osl,
                                                   op0=ALU.mult, op1=ALU.add)
    ppe2.release()
    pe.release()

    # ---------- Phase F: output ----------
    for t in range(NT):
        nc.sync.dma_start(out_ext.ap()[128 * t:128 * (t + 1), :], out_t[t][:])

    for p in (pp2, dram, big, stat, const):
        p.release()


# ---------------- host side ----------------

def prepare_in_maps(c: Cfg, inputs):
    f32 = np.float32
    hs = np.asarray(inputs["hidden_states"], f32)
    ln1_g = np.asarray(inputs["ln1_g"], f32)
    ln1_b = np.asarray(inputs["ln1_b"], f32)
    w_qkv = np.asarray(inputs["w_qkv"], f32)
    b_qkv = np.asarray(inputs["b_qkv"], f32)
    w_dense = np.asarray(inputs["w_dense"], f32)
    b_dense = np.asarray(inputs["b_dense"], f32)
    ln2_g = np.asarray(inputs["ln2_g"], f32)
    ln2_b = np.asarray(inputs["ln2_b"], f32)
    w1 = np.asarray(inputs["w1"], f32)
    b1 = np.asarray(inputs["b1"], f32)
    w2 = np.asarray(inputs["w2"], f32)
    b2 = np.asarray(inputs["b2"], f32)

    H, NH, HD, FF, KH = c.H, c.NH, c.HD, c.FF, c.KH
    NFC, FCT = FF // c.FC, c.FC // 128
    cols = np.concatenate([np.arange(h * 3 * HD, h * 3 * HD + HD)
                           for h in range(NH)])
    wg = ln1_g[:, None] * w_qkv
    wq_f, wk_f, wv_f = wg[:, cols], wg[:, cols + HD], wg[:, cols + 2 * HD]
    bfull = ln1_b @ w_qkv + b_qkv
    bq_f, bk_f, bv_f = bfull[cols], bfull[cols + HD], bfull[cols + 2 * HD]
    bd_f = bv_f @ w_dense + b_dense          # v-bias folded through attention
    w1_f = ln2_g[:, None] * w1
    b1_f = ln2_b @ w1 + b1

    # head-major rows [NH*128, KH*128]: block h = weights for head h
    hmaj = lambda w, nb: np.ascontiguousarray(
        w.reshape(KH, 128, nb, 128).transpose(2, 1, 0, 3)
        .reshape(nb * 128, KH * 128).astype(BF))
    # [128, KH*H]: row p, col kk*H + cc  =  w[kk*128+p, cc]
    parr = lambda w: np.ascontiguousarray(
        w.reshape(KH, 128, H).transpose(1, 0, 2)
        .reshape(128, KH * H).astype(BF))
    wqh = hmaj(wq_f, NH)
    wkh = hmaj(wk_f, NH)
    wvh = parr(wv_f)
    wdh = parr(w_dense)
    w1h = hmaj(w1_f, FF // 128)
    w2h = np.ascontiguousarray(
        w2.reshape(NFC, FCT, 128, H).transpose(0, 2, 1, 3)
        .reshape(NFC * 128, FCT * H).astype(BF))

    inv = 1.0 / (10000.0 ** (np.arange(0, HD, 2, dtype=f32) / HD))
    pos = np.arange(c.S, dtype=f32)
    frq = np.einsum('i,j->ij', pos, inv)
    emb = np.concatenate([frq, frq], axis=-1)
    cos_full = np.cos(emb).T.astype(f32)
    sin_full = np.sin(emb).T.astype(f32)
    sins_full = sin_full.copy()
    sins_full[:HD // 2] *= -1.0

    bf = lambda a: np.ascontiguousarray(a.astype(BF))
    in_maps = []
    for i in range(c.W):
        b, g = i // c.NG, i % c.NG
        t0 = g * c.T
        qpos = np.arange(t0, t0 + c.T)
        # lm-major mask: slot gp = lm*NG + rnk covers key block rnk*NT + lm
        mrows = []
        for gp in range(c.S // 128):
            lm, rnk = gp // c.NG, gp % c.NG
            kb = rnk * c.NT + lm
            kpos = np.arange(kb * 128, (kb + 1) * 128)
            mrows.append((kpos[:, None] <= qpos[None, :]).astype(BF))
        mask = np.concatenate(mrows, axis=0)
        in_maps.append({
            "x": np.ascontiguousarray(hs[b, t0:t0 + c.T, :]),
            "wq": wqh, "wk": wkh, "wv": wvh, "wd": wdh, "w1": w1h, "w2": w2h,
            "bq": np.ascontiguousarray(bq_f.reshape(NH, 128).T.astype(f32)),
            "bk": np.ascontiguousarray(bk_f.reshape(NH, 128).T.astype(f32)),
            "bd": bf(SW * bd_f.reshape(1, H)),
            "b1": np.ascontiguousarray(b1_f.reshape(FF // 128, 128).T
                                       .astype(f32)),
            "b2": bf(SW * b2.reshape(1, H)),
            "ones_r": np.ones((1, c.T), BF),
            "onesdr": np.ones((128, 32), E4),
            "cosT": np.ascontiguousarray(cos_full[:, t0:t0 + c.T]),
            "sinsT": np.ascontiguousarray(sins_full[:, t0:t0 + c.T]),
            "maskT": np.ascontiguousarray(mask),
        })
    return in_maps


def assemble_output(c: Cfg, results):
    out = np.empty((c.B, c.S, c.H), np.float32)
    for i in range(c.W):
        b, g = i // c.NG, i % c.NG
        out[b, g * c.T:(g + 1) * c.T, :] = results[i]["out"]
    return out


def run(nc, c: Cfg, inputs, trace=False, **kw):
    in_maps = prepare_in_maps(c, inputs)
    last = None
    for attempt in range(3):
        try:
            res = bass_utils.run_bass_kernel_spmd(
                nc, in_maps, core_ids=list(range(c.W)), trace=trace, **kw)
            return assemble_output(c, res.results), res
        except Exception as e:
            last = e
            print(f"run attempt {attempt} failed: {type(e).__name__}: {e}",
                  file=sys.stderr)
    raise last


# ======================= harness entry point =======================

_CACHE = {}


def kernel(**inputs):
    """Full-input entry: shard, compile (cached), run on 8 cores, gather."""
    c = Cfg()
    if "nc" not in _CACHE:
        _CACHE["nc"] = build(c)
    out, _ = run(_CACHE["nc"], c, inputs, trace=False)
    return out
